# revision 1
# baseline (speedup 1.0000x reference)
"""Trainium2 Bass kernel for spatial attention (nn_Attention_11407433138897).

Reference computation (B=16, C=512, H=W=32, 4 heads x 128 dim_head):
  qkv = 1x1conv(fmap)                      # [b, 3*512, n],  n = 1024
  sim = (q*scale) @ k^T + (q*scale) @ emb^T
  out = softmax(sim) @ v                   # -> [b, 512, 32, 32]

Key algebraic fold: sim = qs @ (k + emb)^T  -- the positional-bias matmul is
folded into k.  Softmax is computed without max-subtraction (logits ~N(0,1);
exp is safe in fp32/bf16 range).

Distribution: pure data-parallel over batch, 2 batches per NeuronCore, no
collectives.  Matmuls run in bf16 (fp32 PSUM accumulation); q-scale folded
into the weight on the host.

v2 structure (per core), all layouts chosen so no transposes are needed:
  x   [c=512, n=1024]  (c on partitions, 4 chunks)       <- fmap[b]
  wT  [c=512, o=1536]  (host-transposed weight)
  q,k' d-major  [d=128, n]  per head  (k' = k + emb)
  v   n-major   [n, o_v=512]          (v[j,d] = lhsT of PV matmul)
  simT[j, i] = k'^T q   (j on partitions -> PV needs no transpose)

Key v2 changes vs v1 (159 us):
  * Wide 2-bank PSUM tiles [128,1024]: every ACT/DVE consumer (exp, q-copy,
    k-add, v-cast, recip, final mul) processes 1024 elems/lane per
    instruction, halving the per-instruction fixed overhead (~352 cyc on ACT).
  * Softmax denominators: ones[128,128] stationary replicates the j-sum onto
    all 128 partitions -> plain elementwise reciprocal+multiply.  Removes the
    32x32-transpose reciprocal spread and the broadcast matmul entirely.
  * reciprocal_approx_fast (1 custom-DVE op) instead of full-precision
    reciprocal.
  * Software pipelining: batch 1's qkv projection matmul groups are emitted
    as fillers inside batch 0's attention jc-loop (and qk groups of later
    heads inside earlier heads), so the PE never starves while ACT grinds
    through exp in the attention phase.
  * Sum-reduction matmuls interleaved at even jc (operating on DVE pairwise
    partial sums from odd jc); the very last head accumulates exp chunks
    directly so the end-of-kernel chain is short.
  * Output stored bf16 (halves out-DMA), embt loaded bf16.
  * Slim TileContext epilogue (keep the DMA-drain, drop per-engine drains).
"""

import os
import sys

import numpy as np
import ml_dtypes

sys.path.insert(0, "/opt/trn_rl_repo")
sys.path.insert(0, "/root/.axon_site")
sys.path.insert(0, "/root/.axon_site/_ro/trn_rl_repo")
sys.path.insert(0, "/root/.axon_site/_ro/pypackages")

HEADS = 4
D = 128           # dim_head
DIM = 512         # input channels
N = 1024          # 32*32 spatial positions
B = 16
N_CORES = 8
B_PER_CORE = B // N_CORES   # 2
SCALE = D ** -0.5
NH = 512          # half of n (PSUM bank = 512 fp32)
NJ = N // 128     # 8 j-chunks
CC = DIM // 128   # 4 contraction chunks

_BF16 = ml_dtypes.bfloat16

_COMPILED = {}


def _patch_tail_barrier(tile):
    """Slim TileContext epilogue: keep the sync drain (DMA-queue flush gated
    on the global semaphore clock = output integrity), drop the per-engine
    drains, semaphore clears, and second barrier (~4-6us of fixed tail for a
    single top-level context)."""
    from concourse.tile import ScopedClock

    def _drain_and_barrier(self, tick_clock, wait_clock):
        drain_inst = self.nc.sync.drain()
        wait_clock.add_sem_waits(
            drain_inst.ins, ScopedClock({None: tick_clock.global_clock})
        )
        self.nc.all_engine_barrier(sem_only=True)
        popped = self.nc._tile_sem_poison_stack.pop()
        assert popped is self._sem_poison

    tile.TileContext._drain_and_barrier = _drain_and_barrier


def _build():
    """Build + compile the per-core Bass graph (cached)."""
    import concourse.bass as bass
    import concourse.tile as tile
    from concourse import bacc, mybir

    if os.environ.get("KERNEL_SLIM_TAIL", "1") == "1":
        _patch_tail_barrier(tile)

    bf16 = mybir.dt.bfloat16
    f32 = mybir.dt.float32
    AF = mybir.ActivationFunctionType

    nc = bacc.Bacc("TRN2", target_bir_lowering=False, debug=False,
                   num_devices=N_CORES)

    x_dram = nc.dram_tensor("x", [B_PER_CORE, DIM, N], bf16, kind="ExternalInput")
    wt_dram = nc.dram_tensor("wt", [DIM, 3 * DIM], bf16, kind="ExternalInput")
    embt_dram = nc.dram_tensor("embt", [D, N], bf16, kind="ExternalInput")
    out_dram = nc.dram_tensor("out", [B_PER_CORE, HEADS * D, N], bf16,
                              kind="ExternalOutput")

    with tile.TileContext(nc) as tc:
        with (
            tc.tile_pool(name="const", bufs=1) as const_pool,
            tc.tile_pool(name="xin", bufs=1) as x_pool,
            tc.tile_pool(name="qkv", bufs=1) as qkv_pool,
            tc.tile_pool(name="expsim", bufs=6) as exp_pool,
            tc.tile_pool(name="padd", bufs=4) as padd_pool,
            tc.tile_pool(name="rec", bufs=3) as rec_pool,
            tc.tile_pool(name="outsb", bufs=3) as out_pool,
            tc.tile_pool(name="wide_ps", bufs=2, space="PSUM") as wide_ps,
            tc.tile_pool(name="pv_ps", bufs=2, space="PSUM") as pv_ps,
        ):
            # ---- input DMAs first: x[b0] interleaved across sync+scalar
            # queues (even c on sync, odd on scalar) so the first v-group's
            # chunks land ASAP.  wtv/wtq on gpsimd.  x[b1] behind on sync. ----
            x_sb = [[[x_pool.tile([128, NH], bf16, tag=f"x{b}_{c}_{nh}",
                                  name=f"x{b}_{c}_{nh}")
                      for nh in range(2)] for c in range(CC)]
                    for b in range(B_PER_CORE)]
            for nh in range(2):
                for c in range(CC):
                    eng = nc.sync if c % 2 == 0 else nc.scalar
                    eng.dma_start(x_sb[0][c][nh][:],
                                  x_dram[0, bass.ts(c, 128), bass.ts(nh, NH)])
            wtv_sb = [const_pool.tile([128, DIM], bf16, tag=f"wtv{c}",
                                      name=f"wtv{c}") for c in range(CC)]
            wtq_sb = [const_pool.tile([128, DIM], bf16, tag=f"wtq{c}",
                                      name=f"wtq{c}") for c in range(CC)]
            wtk_sb = [const_pool.tile([128, DIM], bf16, tag=f"wtk{c}",
                                      name=f"wtk{c}") for c in range(CC)]
            for c in range(CC):
                nc.gpsimd.dma_start(wtv_sb[c][:],
                                    wt_dram[bass.ts(c, 128), 2 * DIM:3 * DIM])
            for c in range(CC):
                nc.gpsimd.dma_start(wtq_sb[c][:], wt_dram[bass.ts(c, 128), 0:DIM])

            # ---- constants (memsets first on vector so the PE warm-up
            # isn't stuck behind DMA-issue occupancy) ----
            warm_sb = const_pool.tile([128, NH], bf16, tag="warm")
            nc.vector.memset(warm_sb[:], 1.0)
            ones128 = const_pool.tile([128, 128], bf16, tag="ones128")
            nc.vector.memset(ones128[:], 1.0)

            for c in range(CC):
                nc.scalar.dma_start(wtk_sb[c][:],
                                    wt_dram[bass.ts(c, 128), DIM:2 * DIM])
            embt_sb = const_pool.tile([D, N], bf16, tag="embt")
            nc.scalar.dma_start(embt_sb[:], embt_dram[:])
            for c in range(CC):
                for nh in range(2):
                    nc.sync.dma_start(x_sb[1][c][nh][:],
                                      x_dram[1, bass.ts(c, 128),
                                             bass.ts(nh, NH)])

            # preload the exp table set on ACT during the DMA wait (~2.7us
            # one-time ACT_TABLE_LOAD would otherwise land on the first
            # real exp mid-kernel)
            exp_warm = const_pool.tile([1, 8], bf16, tag="exp_warm")
            nc.scalar.activation(exp_warm[:], warm_sb[0:1, 0:8], AF.Exp)

            # ---- PE warm-up: junk matmuls while input DMAs are in flight;
            # flips the HAM clock gate toward 2.4 GHz before real work ----
            warm_ps = wide_ps.tile([128, 2 * NH], f32, tag="w", name="warm_ps")
            for i in range(10):
                nc.tensor.matmul(warm_ps[:, bass.ts(i % 2, NH)],
                                 warm_sb[:, 0:128], warm_sb[:],
                                 start=True, stop=True)
            warm_out = const_pool.tile([1, 8], f32, tag="warm_out")
            nc.vector.tensor_copy(warm_out[:], warm_ps[0:1, 0:8])
            warm_dram = nc.dram_tensor("warm_scratch", [1, 8], f32)
            nc.scalar.dma_start(warm_dram[:], warm_out[:])

            # ---- qkv staging (per-batch tags; no WAR serialization) ----
            q_sb = [qkv_pool.tile([128, HEADS * N], bf16, tag=f"q{b}",
                                  name=f"q{b}") for b in range(B_PER_CORE)]
            k_sb = [qkv_pool.tile([128, HEADS * N], bf16, tag=f"k{b}",
                                  name=f"k{b}") for b in range(B_PER_CORE)]
            v_sb = [qkv_pool.tile([128, NJ * DIM], bf16, tag=f"v{b}",
                                  name=f"v{b}") for b in range(B_PER_CORE)]

            # ---- projection group emitters (each: one wide PSUM tile,
            # 8 accumulating matmuls, one wide DVE consumer) ----
            def emit_qk_group(b, h, which):
                ps = wide_ps.tile([128, 2 * NH], f32, tag="w",
                                  name=f"{which}{b}_{h}")
                wt_t = wtq_sb if which == "q" else wtk_sb
                for c in range(CC):
                    for nh in range(2):
                        nc.tensor.matmul(
                            ps[:, bass.ts(nh, NH)],
                            wt_t[c][:, bass.ts(h, 128)],
                            x_sb[b][c][nh][:],
                            start=(c == 0), stop=(c == CC - 1),
                        )
                if which == "q":
                    nc.vector.tensor_copy(q_sb[b][:, h * N:(h + 1) * N], ps[:])
                else:
                    nc.vector.tensor_add(k_sb[b][:, h * N:(h + 1) * N],
                                         ps[:], embt_sb[:])

            def emit_v_group(b, g):
                # covers j-chunks 2g, 2g+1 -> v_sb cols [g*1024, (g+1)*1024)
                ps = wide_ps.tile([128, 2 * NH], f32, tag="w", name=f"v{b}_{g}")
                for c in range(CC):
                    for jo in range(2):
                        j = 2 * g + jo
                        nc.tensor.matmul(
                            ps[:, bass.ts(jo, NH)],
                            x_sb[b][c][j // 4][:, bass.ts(j % 4, 128)],
                            wtv_sb[c][:],
                            start=(c == 0), stop=(c == CC - 1),
                        )
                nc.vector.tensor_copy(v_sb[b][:, bass.ts(g, 2 * NH)], ps[:])

            # ---- attention for one head; fillers pulled from a queue and
            # emitted between jc steps (Bresenham pacing over the window).
            # Softmax denominators: pairwise exp sums reduced by a DVE adder
            # tree (4 padds -> 2 -> 1), so only ONE ones-matmul pair per head
            # does the partition reduction.  That pair plus recip/mul/DMA is
            # DEFERRED into the next head's jc-loop so the long DVE chain
            # never stalls the PE FIFO at a head boundary.  The very last
            # head instead accumulates exp chunks directly (short tail) into
            # a transient wide tile from the pv rotation. ----
            def emit_attn_head(b, h, fillers, state, window, last, deferred):
                q_h = q_sb[b][:, h * N:(h + 1) * N]
                k_h = k_sb[b][:, h * N:(h + 1) * N]
                pv = pv_ps.tile([128, 2 * NH], f32, tag="pv",
                                name=f"pv{b}_{h}")
                sums_w = None
                exs = [None] * NJ
                padds = []

                def drain_fillers():
                    state[0] += 1
                    while (len(fillers) and
                           state[1] < (state[0] * state[2] + window - 1) // window):
                        fillers.pop(0)()
                        state[1] += 1

                for jc in range(NJ):
                    if deferred is not None and jc == (0 if last else 3):
                        deferred()
                        deferred = None
                    sim = wide_ps.tile([128, 2 * NH], f32, tag="w",
                                       name=f"sim{b}_{h}_{jc}")
                    for ih in range(2):
                        nc.tensor.matmul(
                            sim[:, bass.ts(ih, NH)],
                            k_h[:, bass.ts(jc, 128)],
                            q_h[:, bass.ts(ih, NH)],
                            start=True, stop=True,
                        )
                    ex = exp_pool.tile([128, 2 * NH], bf16, tag="exp",
                                       name=f"ex{b}_{h}_{jc}")
                    if last and jc == NJ - 1:
                        # split final exp: the ih0 tail chain unblocks after
                        # 720ns instead of 1113ns
                        for ih in range(2):
                            nc.scalar.activation(ex[:, bass.ts(ih, NH)],
                                                 sim[:, bass.ts(ih, NH)],
                                                 AF.Exp)
                    else:
                        nc.scalar.activation(ex[:], sim[:], AF.Exp)
                    exs[jc] = ex
                    # pv lags one jc so its LDWEIGHTS prefetches during the
                    # sim stream instead of serializing after the sem wait
                    if jc > 0:
                        for ih in range(2):
                            nc.tensor.matmul(
                                pv[:, bass.ts(ih, NH)],
                                v_sb[b][:, (jc - 1) * NH + h * 128:
                                        (jc - 1) * NH + h * 128 + 128],
                                exs[jc - 1][:, bass.ts(ih, NH)],
                                start=(jc == 1), stop=False,
                            )
                    if last:
                        # semi-direct: padd for ex0+ex1, then direct pairs
                        if jc == 1:
                            pa = padd_pool.tile([128, 2 * NH], bf16,
                                                tag="padd",
                                                name=f"pa{b}_{h}_0")
                            nc.vector.tensor_add(pa[:], exs[0][:], exs[1][:])
                            padds.append(pa)
                        elif jc == 2:
                            sums_w = pv_ps.tile([128, 2 * NH], f32, tag="pv",
                                                name=f"sums{b}_{h}")
                            for ih in range(2):
                                nc.tensor.matmul(
                                    sums_w[:, bass.ts(ih, NH)], ones128[:],
                                    padds[0][:, bass.ts(ih, NH)],
                                    start=True, stop=False,
                                )
                        elif jc >= 3:
                            for ih in range(2):
                                nc.tensor.matmul(
                                    sums_w[:, bass.ts(ih, NH)], ones128[:],
                                    exs[jc - 1][:, bass.ts(ih, NH)],
                                    start=False, stop=False,
                                )
                    else:
                        if jc % 2 == 1:
                            pa = padd_pool.tile([128, 2 * NH], bf16,
                                                tag="padd",
                                                name=f"pa{b}_{h}_{jc // 2}")
                            nc.vector.tensor_add(pa[:], exs[jc - 1][:],
                                                 exs[jc][:])
                            padds.append(pa)
                        if jc == 4:
                            pa2 = padd_pool.tile([128, 2 * NH], bf16,
                                                 tag="padd2",
                                                 name=f"pa2{b}_{h}_0")
                            nc.vector.tensor_add(pa2[:], padds[0][:],
                                                 padds[1][:])
                            padds.append(pa2)   # padds[2] slot shifts below
                    drain_fillers()

                if not last:
                    # final pv pair (j-chunk NJ-1)
                    for ih in range(2):
                        nc.tensor.matmul(
                            pv[:, bass.ts(ih, NH)],
                            v_sb[b][:, (NJ - 1) * NH + h * 128:
                                    (NJ - 1) * NH + h * 128 + 128],
                            exs[NJ - 1][:, bass.ts(ih, NH)],
                            start=False, stop=True,
                        )
                if last:
                    # final direct sum contribution + immediate normalize,
                    # per-ih interleaved (sums then pv per half) so recip0
                    # fires after two MMs; halves pipelined across
                    # sync/scalar DMA queues
                    for ih in range(2):
                        nc.tensor.matmul(
                            sums_w[:, bass.ts(ih, NH)], ones128[:],
                            exs[NJ - 1][:, bass.ts(ih, NH)],
                            start=False, stop=True,
                        )
                        nc.tensor.matmul(
                            pv[:, bass.ts(ih, NH)],
                            v_sb[b][:, (NJ - 1) * NH + h * 128:
                                    (NJ - 1) * NH + h * 128 + 128],
                            exs[NJ - 1][:, bass.ts(ih, NH)],
                            start=False, stop=True,
                        )
                    rec = rec_pool.tile([128, 2 * NH], f32, tag="rec",
                                        name=f"rec{b}_{h}")
                    o = out_pool.tile([128, 2 * NH], bf16, tag="o",
                                      name=f"o{b}_{h}")
                    for ih in range(2):
                        nc.vector.reciprocal_approx_fast(
                            out=rec[:, bass.ts(ih, NH)],
                            in_=sums_w[:, bass.ts(ih, NH)])
                        nc.vector.tensor_mul(o[:, bass.ts(ih, NH)],
                                             pv[:, bass.ts(ih, NH)],
                                             rec[:, bass.ts(ih, NH)])
                        eng = nc.sync if ih == 0 else nc.scalar
                        eng.dma_start(
                            out_dram[b, h * D:(h + 1) * D, bass.ts(ih, NH)],
                            o[:, bass.ts(ih, NH)])
                    return None

                # tree levels 2+3 (consumed by the deferred finisher)
                pa2_1 = padd_pool.tile([128, 2 * NH], bf16, tag="padd2",
                                       name=f"pa2{b}_{h}_1")
                nc.vector.tensor_add(pa2_1[:], padds[3][:], padds[4][:])
                pa3 = padd_pool.tile([128, 2 * NH], bf16, tag="padd3",
                                     name=f"pa3{b}_{h}")
                nc.vector.tensor_add(pa3[:], padds[2][:], pa2_1[:])

                def finish():
                    sums = wide_ps.tile([128, 2 * NH], f32, tag="w",
                                        name=f"sums{b}_{h}")
                    for ih in range(2):
                        nc.tensor.matmul(
                            sums[:, bass.ts(ih, NH)], ones128[:],
                            pa3[:, bass.ts(ih, NH)],
                            start=True, stop=True,
                        )
                    rec = rec_pool.tile([128, 2 * NH], f32, tag="rec",
                                        name=f"rec{b}_{h}")
                    nc.vector.reciprocal_approx_fast(out=rec[:], in_=sums[:])
                    o = out_pool.tile([128, 2 * NH], bf16, tag="o",
                                      name=f"o{b}_{h}")
                    nc.vector.tensor_mul(o[:], pv[:], rec[:])
                    nc.sync.dma_start(out_dram[b, h * D:(h + 1) * D, :], o[:])

                return finish

            # ---- program order ----
            # batch 0 qkv head-0 prerequisites
            for g in range(4):
                emit_v_group(0, g)
            emit_qk_group(0, 0, "k")
            emit_qk_group(0, 0, "q")

            # batch 0 attention; fillers: rest of b0 qk, all of b1 v/qk-h0
            fillers0 = []
            for h in range(1, HEADS):
                fillers0.append(lambda h=h: emit_qk_group(0, h, "k"))
                fillers0.append(lambda h=h: emit_qk_group(0, h, "q"))
            for g in range(4):
                fillers0.append(lambda g=g: emit_v_group(1, g))
            fillers0.append(lambda: emit_qk_group(1, 0, "k"))
            fillers0.append(lambda: emit_qk_group(1, 0, "q"))
            state0 = [0, 0, len(fillers0)]   # [steps, emitted, total]
            deferred = None
            for h in range(HEADS):
                deferred = emit_attn_head(0, h, fillers0, state0, window=20,
                                          last=False, deferred=deferred)
            while fillers0:
                fillers0.pop(0)()

            # batch 1 attention; fillers: b1 qk heads 1-3 over first 3 heads
            fillers1 = []
            for h in range(1, HEADS):
                fillers1.append(lambda h=h: emit_qk_group(1, h, "k"))
                fillers1.append(lambda h=h: emit_qk_group(1, h, "q"))
            state1 = [0, 0, len(fillers1)]
            for h in range(HEADS):
                deferred = emit_attn_head(1, h, fillers1, state1, window=10,
                                          last=(h == HEADS - 1),
                                          deferred=deferred)
            while fillers1:
                fillers1.pop(0)()

    nc.compile()
    return nc


def _get_compiled():
    if "nc" not in _COMPILED:
        _COMPILED["nc"] = _build()
    return _COMPILED["nc"]


def _run(fmap, w_qkv, emb_h, emb_w, **spmd_kwargs):
    from concourse.bass_utils import run_bass_kernel_spmd

    nc = _get_compiled()

    fmap = np.asarray(fmap, dtype=np.float32)
    w_qkv = np.asarray(w_qkv, dtype=np.float32)
    emb_h = np.asarray(emb_h, dtype=np.float32)
    emb_w = np.asarray(emb_w, dtype=np.float32)

    b, c, hh, ww = fmap.shape
    x = fmap.reshape(b, c, hh * ww)

    # fold q scale into weight rows, transpose to [c, o], cast to bf16
    w = w_qkv.copy()
    w[:HEADS * D] *= SCALE
    wt = np.ascontiguousarray(w.T).astype(_BF16)

    embt = np.ascontiguousarray(
        (emb_h[:, None, :] + emb_w[None, :, :]).reshape(N, D).T
    ).astype(_BF16)

    x16 = x.astype(_BF16)
    in_maps = [
        {
            "x": np.ascontiguousarray(x16[i * B_PER_CORE:(i + 1) * B_PER_CORE]),
            "wt": wt,
            "embt": embt,
        }
        for i in range(N_CORES)
    ]

    res = run_bass_kernel_spmd(nc, in_maps, core_ids=list(range(N_CORES)),
                               **spmd_kwargs)
    out = np.concatenate(
        [np.asarray(res.results[i]["out"], dtype=np.float32)
         for i in range(N_CORES)], axis=0)
    return out.reshape(B, HEADS * D, hh, ww), res


def kernel(fmap, w_qkv, emb_h, emb_w):
    out, _ = _run(fmap, w_qkv, emb_h, emb_w)
    return out


if __name__ == "__main__":
    rng = np.random.default_rng(0)
    fmap = rng.standard_normal((B, DIM, 32, 32), dtype=np.float32)
    w_qkv = rng.standard_normal((3 * HEADS * D, DIM), dtype=np.float32) * DIM ** -0.5
    emb_h = rng.standard_normal((32, D), dtype=np.float32) * SCALE
    emb_w = rng.standard_normal((32, D), dtype=np.float32) * SCALE
    out = kernel(fmap=fmap, w_qkv=w_qkv, emb_h=emb_h, emb_w=emb_w)
    print("kernel out:", out.shape, out.dtype)

